# revision 41
# baseline (speedup 1.0000x reference)
"""Trainium2 Bass kernel for nn_AdversarialHead (scatter_memory).

Computes, for F*A = 131072 rows:
  one_hot  = multi-hot(actions + starts)                      [rows, 21]
  h        = leaky_relu(fc1_w @ [cur; one_hot] + fc1_b, 0.1)  [rows, 140]
  pred     = fc2_w @ h + fc2_b                                [rows, 128]
  logits_i = head_wi @ [cur; nxt] + head_bi                   [rows, 5/7/9]

Strategy (pure data parallel over 8 cores, 16384 rows each):
- Host pre-transposes cur/nxt to [feat, rows] so every device matmul is a
  plain orientation-B matmul (features on partitions, rows on the free dim).
- The one-hot scatter becomes: per-partition is_equal against an iota
  constant (building one-hotT tiles on chip), then a small-K matmul against
  host-prebuilt embedding tables (fc1_w columns re-indexed by action value).
  All biases are folded into the row-15 "ones" row of those tables.
- leaky_relu is decomposed as 0.1*x + 0.9*relu(x); the linear 0.1*x path is
  folded into the weights (Wfuse = 0.1*fc2_w@fc1_w) so the device only needs
  a single Relu per PSUM tile; the relu path uses 0.9-scaled fc2 weights.
- f32 mode uses float32r matmuls (full-rate PE); bf16 mode halves HBM bytes.

Outputs are produced transposed ([feat, rows]) and un-transposed on host.
"""
import os
import sys
import types

import numpy as np
import ml_dtypes

import concourse.bass as bass
import concourse.tile as tile
from concourse import bacc, mybir
from concourse.bass_utils import run_bass_kernel_spmd

# If BASS_TRACE is set in the environment, run_bass_kernel_spmd imports
# antenv.axon_hooks, which this image lacks. Provide a no-op hook module so
# tracing degrades gracefully instead of raising ImportError.
if "antenv.axon_hooks" not in sys.modules:
    try:
        import antenv.axon_hooks  # noqa: F401
    except ImportError:
        _m = types.ModuleType("antenv.axon_hooks")
        _m.get_axon_ntff_profile_hook = lambda: None
        _m.set_axon_ntff_profile_hook = lambda h: None
        sys.modules["antenv.axon_hooks"] = _m

# ---------------------------------------------------------------- constants
F_FRAMES, A_AGENTS, FEAT = 4096, 32, 128
NVEC = [5, 7, 9]
STARTS = [0, 5, 12]
ACT_TOTAL = 21            # sum(NVEC)
HID = 140
NCORES = 8
ROWS = F_FRAMES * A_AGENTS          # 131072
RPC = ROWS // NCORES                # 16384 rows per core
TILE_N = 1024                       # rows per megatile (2 PSUM banks)
NT = RPC // TILE_N                  # 16 megatiles per core
HID2 = HID - FEAT                   # 12 hidden dims beyond partition 128

# psum2 layout: rows 0:21 = logits, rows 21:32 = zero pad, rows 32:44 = h2
# (compute-engine PSUM reads must start at a 32-aligned partition)
NL = 44
H2_BASE = 32

# Variant: "bf16" (default, ~4e-3 scale-relative error),
#          "f32"  (float32 I/O, float32r matmuls — fast but pred path is
#                  less accurate than bf16 in absmax),
#          "f32x" (plain float32 matmuls — exact but PE runs at 1/4 rate)
VARIANT = os.environ.get("KERNEL_DTYPE", "bf16")

_prog_cache = {}


def _np_dt(variant):
    return ml_dtypes.bfloat16 if variant == "bf16" else np.float32


def _prep_weights(fc1_w, fc1_b, fc2_w, fc2_b,
                  head_w0, head_b0, head_w1, head_b1, head_w2, head_b2):
    """Build all stationary (lhsT) operands in float32. Shapes noted as [K, M]."""
    f32 = np.float32
    fc1_w = np.asarray(fc1_w, f32); fc1_b = np.asarray(fc1_b, f32)
    fc2_w = np.asarray(fc2_w, f32); fc2_b = np.asarray(fc2_b, f32)
    wcat = np.concatenate([np.asarray(head_w0, f32), np.asarray(head_w1, f32),
                           np.asarray(head_w2, f32)], axis=0)          # [21, 256]
    bcat = np.concatenate([np.asarray(head_b0, f32), np.asarray(head_b1, f32),
                           np.asarray(head_b2, f32)], axis=0)          # [21]

    wfuse = 0.1 * (fc2_w @ fc1_w)                                      # [128, 149]

    # embedding row index map: p = t*5 + v  ->  fc1 input column 128+STARTS[t]+v
    emb_cols = [FEAT + STARTS[t] + v for t in range(3) for v in range(5)]

    # Every lhsT is zero-padded to [128, 128]: uniform full-array matmuls
    # keep the PE pipeline dense (small-K / partial-M matmuls measured 2.8x
    # slower and prevent the clock gate from warming). Zero weight rows make
    # the padded contraction exact as long as the rhs rows are finite.
    w = {k: np.zeros((FEAT, FEAT), f32) for k in
         ("wa", "wb", "wc", "wd", "we", "wi", "wj", "wf2", "wg2")}
    w["wa"][:, :] = fc1_w[:FEAT, :FEAT].T               # h1 from cur
    w["wb"][:, :ACT_TOTAL] = wcat[:, :FEAT].T           # logits+h2 from cur
    w["wb"][:, H2_BASE:H2_BASE + HID2] = fc1_w[FEAT:, :FEAT].T
    w["wc"][:15, :] = fc1_w[:FEAT, emb_cols].T          # h1 from one-hot
    w["wc"][15, :] = fc1_b[:FEAT]
    w["wd"][:15, H2_BASE:H2_BASE + HID2] = fc1_w[FEAT:, emb_cols].T
    w["wd"][15, H2_BASE:H2_BASE + HID2] = fc1_b[FEAT:]
    w["wd"][15, :ACT_TOTAL] = bcat                      # h2 + all biases
    w["we"][:, :ACT_TOTAL] = wcat[:, FEAT:].T           # logits from nxt
    w["wi"][:, :] = wfuse[:, :FEAT].T                   # 0.1-path from cur
    w["wj"][:15, :] = wfuse[:, emb_cols].T              # 0.1-path from one-hot
    w["wj"][15, :] = 0.1 * (fc2_w @ fc1_b) + fc2_b
    w["wf2"][:, :] = (0.9 * fc2_w[:, :FEAT]).T          # relu path h1
    w["wg2"][H2_BASE:H2_BASE + HID2, :] = (0.9 * fc2_w[:, FEAT:]).T
    return w


def _build_program(variant):
    """Build the SPMD Bass program (identical across cores)."""
    if variant == "f32":
        dt_dat = mybir.dt.float32r     # matmul operands, stored as f32
        dt_out = mybir.dt.float32
    elif variant == "f32x":
        dt_dat = mybir.dt.float32
        dt_out = mybir.dt.float32
    else:
        dt_dat = mybir.dt.bfloat16
        dt_out = mybir.dt.bfloat16
    dt_act = mybir.dt.bfloat16         # action values (0..4, exact in bf16)
    f32 = mybir.dt.float32

    AF = mybir.ActivationFunctionType
    OP = mybir.AluOpType

    nc = bacc.Bacc(None, target_bir_lowering=False, debug=False)

    def din(name, shape, dt):
        return nc.dram_tensor(name, list(shape), dt, kind="ExternalInput").ap()

    def dout(name, shape, dt):
        return nc.dram_tensor(name, list(shape), dt, kind="ExternalOutput").ap()

    curT = din("curT", (FEAT, RPC), dt_dat)
    nxtT = din("nxtT", (FEAT, RPC), dt_dat)
    act16 = din("act16", (16, RPC), dt_act)
    w_names = ("wa", "wb", "wc", "wd", "we", "wi", "wj", "wf2", "wg2")
    w_all = din("w_all", (FEAT, FEAT * len(w_names)), dt_dat)
    iota128 = din("iota128", (128, 1), f32)
    predT = dout("predT", (FEAT, RPC), dt_out)
    logitsT = dout("logitsT", (ACT_TOTAL, RPC), f32)

    with tile.TileContext(nc) as tc:
        with (
            tc.tile_pool(name="consts", bufs=1) as cp,
            tc.tile_pool(name="io", bufs=4) as iop,
            tc.tile_pool(name="mid", bufs=3) as mp,
            tc.tile_pool(name="pp1", bufs=1, space="PSUM") as pp1,
            tc.tile_pool(name="pp2", bufs=1, space="PSUM") as pp2,
            tc.tile_pool(name="pp3", bufs=2, space="PSUM") as pp3,
        ):
            # ---- load constants once: scalar+gpsimd only, so the sync
            # engine can start the first tile's input DMAs immediately ----
            # early copy of just wa+wc (64KB) so tile 0's first matmuls
            # aren't gated on the full 288KB weight pack landing
            w0_t = cp.tile([FEAT, 3 * FEAT], dt_dat, name="c_w0")
            nc.scalar.dma_start(w0_t[0:64, :], w_all[0:64, 0:3 * FEAT])
            nc.gpsimd.dma_start(w0_t[64:128, :], w_all[64:128, 0:3 * FEAT])
            w_t = cp.tile([FEAT, FEAT * len(w_names)], dt_dat, name="c_wall")
            for q in range(4):
                ps_ = slice(q * 32, (q + 1) * 32)
                eng = nc.scalar if q % 2 == 0 else nc.gpsimd
                eng.dma_start(w_t[ps_, :], w_all[ps_, :])
            cw = {k: w_t[:, i * FEAT:(i + 1) * FEAT]
                  for i, k in enumerate(w_names)}
            cw0 = {"wa": w0_t[:, 0:FEAT], "wb": w0_t[:, FEAT:2 * FEAT],
                   "wc": w0_t[:, 2 * FEAT:3 * FEAT]}
            iota_t = cp.tile([128, 1], f32, name="c_iota")
            nc.scalar.dma_start(iota_t[:], iota128)

            # variable tile schedule: small tiles at both ends shorten the
            # serial fill (first input transfer) and tail (last output chain)
            SCHED = [512, 512] + [1024] * 14 + [512, 512]
            NTV = len(SCHED)

            def emit_stage1(it, off, n):
                """input DMAs, one-hot build, and the 7 stage-1 matmuls"""
                sl = slice(off, off + n)
                cur_t = iop.tile([FEAT, n], dt_dat, tag="cur")
                if it == 0:
                    for q in range(4):
                        ps_ = slice(q * 32, (q + 1) * 32)
                        eng = nc.sync if q % 2 == 0 else nc.gpsimd
                        eng.dma_start(cur_t[ps_, :], curT[ps_, sl])
                else:
                    nc.sync.dma_start(cur_t[0:64, :], curT[0:64, sl])
                    nc.gpsimd.dma_start(cur_t[64:128, :], curT[64:128, sl])
                nxt_t = iop.tile([FEAT, n], dt_dat, tag="nxt")
                nc.gpsimd.dma_start(nxt_t[0:64, :], nxtT[0:64, sl])
                nc.sync.dma_start(nxt_t[64:128, :], nxtT[64:128, sl])
                act_t = iop.tile([128, n], dt_act, tag="act")
                nc.sync.dma_start(act_t[0:16, :], act16[:, sl])

                # one-hotT tile; only rows 0:16 are loaded — rows 16:127
                # compare stale SBUF data, but is_equal always yields finite
                # 0/1 and those rows hit zero-padded weight rows, so the
                # matmul contribution is exactly zero.
                oh = mp.tile([128, n], dt_dat, tag="oh")
                nc.vector.tensor_scalar(oh[:], act_t[:], iota_t[:], None,
                                        OP.is_equal)

                p1 = pp1.tile([FEAT, n], f32, tag="p1")   # h1 pre-act
                p2 = pp2.tile([FEAT, n], f32, tag="p2")   # logits + h2
                p3 = pp3.tile([FEAT, n], f32, tag="p3")   # pred accum
                # weight-major order: each stationary is used for both
                # 512-halves back-to-back so the weight load amortizes
                halves = [slice(s * 512, min((s + 1) * 512, n))
                          for s in range(max(1, n // 512))]
                wsrc = {**cw, **cw0} if it == 0 else cw
                for w_key, rhs, psum, st_, sp_ in (
                        ("wa", cur_t, p1, True, False),
                        ("wc", oh, p1, False, True),
                        ("wb", cur_t, p2, True, False),
                        ("wd", oh, p2, False, False),
                        ("we", nxt_t, p2, False, True),
                        ("wi", cur_t, p3, True, False),
                        ("wj", oh, p3, False, False)):
                    for hs in halves:
                        nc.tensor.matmul(psum[:, hs], wsrc[w_key], rhs[:, hs],
                                         start=st_, stop=sp_)

                h1 = mp.tile([FEAT, n], dt_dat, tag="h1")
                nc.scalar.activation(h1[:], p1[:], AF.Relu)
                hl2 = mp.tile([FEAT, n], dt_dat, tag="hl2")
                nc.scalar.activation(hl2[:], p2[:], AF.Relu)
                lg = mp.tile([ACT_TOTAL, n], f32, tag="lg")
                nc.vector.tensor_copy(lg[:], p2[0:ACT_TOTAL, :])
                return {"sl": sl, "p3": p3, "h1": h1, "hl2": hl2, "lg": lg,
                        "it": it, "n": n}

            def emit_stage2(st):
                """relu-path matmuls + pred/logits drain for a prior tile.

                Emitted after the NEXT tile's stage-1 matmuls so the PE never
                stalls waiting for this tile's Relu outputs (head-of-line
                blocking on the in-order PE queue)."""
                it, sl, p3 = st["it"], st["sl"], st["p3"]
                halves = [slice(s * 512, min((s + 1) * 512, st["n"]))
                          for s in range(max(1, st["n"] // 512))]
                for hs in halves:
                    nc.tensor.matmul(p3[:, hs], cw["wf2"], st["h1"][:, hs],
                                     start=False, stop=False)
                for hs in halves:
                    nc.tensor.matmul(p3[:, hs], cw["wg2"], st["hl2"][:, hs],
                                     start=False, stop=True)
                pred = mp.tile([FEAT, st["n"]], dt_out, tag="pred")
                nc.vector.tensor_copy(pred[:], p3[:])
                if it >= NTV - 2:
                    # last tiles: 4-way split so the tail DMA flush is short
                    nc.gpsimd.dma_start(predT[0:32, sl], pred[0:32, :])
                    nc.sync.dma_start(predT[32:64, sl], pred[32:64, :])
                    nc.gpsimd.dma_start(predT[64:96, sl], pred[64:96, :])
                    nc.sync.dma_start(predT[96:128, sl], pred[96:128, :])
                else:
                    nc.gpsimd.dma_start(predT[0:64, sl], pred[0:64, :])
                    nc.sync.dma_start(predT[64:128, sl], pred[64:128, :])
                if it >= NTV - 2:
                    nc.sync.dma_start(logitsT[:, sl], st["lg"][:])
                else:
                    nc.gpsimd.dma_start(logitsT[:, sl], st["lg"][:])

            prev = None
            off = 0
            for it, n in enumerate(SCHED):
                st = emit_stage1(it, off, n)
                off += n
                if prev is not None:
                    emit_stage2(prev)
                prev = st
            emit_stage2(prev)

    nc.compile()
    return nc


def _get_program(variant):
    if variant not in _prog_cache:
        _prog_cache[variant] = _build_program(variant)
    return _prog_cache[variant]


def _prep_inputs(inputs, variant):
    """Host-side data staging: transpose/shard/cast. Returns per-core in_maps."""
    npdt = _np_dt(variant)
    cur = np.asarray(inputs["current_feature"], np.float32).reshape(ROWS, FEAT)
    nxt = np.asarray(inputs["next_feature"], np.float32).reshape(ROWS, FEAT)
    acts = np.asarray(inputs["actions"])                       # [F, 3, A] int32

    curT = np.ascontiguousarray(cur.T).astype(npdt)            # [128, ROWS]
    nxtT = np.ascontiguousarray(nxt.T).astype(npdt)
    a3 = np.transpose(acts, (1, 0, 2)).reshape(3, ROWS)        # row t = type t
    act16 = np.zeros((16, ROWS), ml_dtypes.bfloat16)
    act16[:15] = np.repeat(a3, 5, axis=0).astype(ml_dtypes.bfloat16)

    w = _prep_weights(
        inputs["fc1_w"], inputs["fc1_b"], inputs["fc2_w"], inputs["fc2_b"],
        inputs["head_w0"], inputs["head_b0"], inputs["head_w1"],
        inputs["head_b1"], inputs["head_w2"], inputs["head_b2"])
    w_names = ("wa", "wb", "wc", "wd", "we", "wi", "wj", "wf2", "wg2")
    w_all = np.ascontiguousarray(
        np.concatenate([w[k] for k in w_names], axis=1)).astype(npdt)
    w = {"w_all": w_all}
    iota128 = np.ascontiguousarray(
        np.tile(np.array([0, 1, 2, 3, 4] * 3 + [0], np.float32), 8)
    ).reshape(128, 1)

    in_maps = []
    for c in range(NCORES):
        s = slice(c * RPC, (c + 1) * RPC)
        m = {"curT": np.ascontiguousarray(curT[:, s]),
             "nxtT": np.ascontiguousarray(nxtT[:, s]),
             "act16": np.ascontiguousarray(act16[:, s]),
             "iota128": iota128}
        m.update(w)
        in_maps.append(m)
    return in_maps


def _assemble_outputs(results):
    predT = np.concatenate([np.asarray(r["predT"], np.float32)
                            for r in results], axis=1)          # [128, ROWS]
    logitsT = np.concatenate([np.asarray(r["logitsT"], np.float32)
                              for r in results], axis=1)        # [21, ROWS]
    pred = predT.T.reshape(F_FRAMES, A_AGENTS, FEAT).astype(np.float32)
    lg = logitsT.T                                              # [ROWS, 21]
    logits0 = lg[:, 0:5].reshape(F_FRAMES, A_AGENTS, 5).astype(np.float32)
    logits1 = lg[:, 5:12].reshape(F_FRAMES, A_AGENTS, 7).astype(np.float32)
    logits2 = lg[:, 12:21].reshape(F_FRAMES, A_AGENTS, 9).astype(np.float32)
    return pred, logits0, logits1, logits2


def kernel(**inputs):
    variant = VARIANT
    nc = _get_program(variant)
    in_maps = _prep_inputs(inputs, variant)
    res = run_bass_kernel_spmd(nc, in_maps, list(range(NCORES)))
    return _assemble_outputs(res.results)


# revision 42
# speedup vs baseline: 1.0149x; 1.0149x over previous
"""Trainium2 Bass kernel for nn_AdversarialHead (scatter_memory).

Computes, for F*A = 131072 rows:
  one_hot  = multi-hot(actions + starts)                      [rows, 21]
  h        = leaky_relu(fc1_w @ [cur; one_hot] + fc1_b, 0.1)  [rows, 140]
  pred     = fc2_w @ h + fc2_b                                [rows, 128]
  logits_i = head_wi @ [cur; nxt] + head_bi                   [rows, 5/7/9]

Strategy (pure data parallel over 8 cores, 16384 rows each):
- Host pre-transposes cur/nxt to [feat, rows] so every device matmul is a
  plain orientation-B matmul (features on partitions, rows on the free dim).
- The one-hot scatter becomes: per-partition is_equal against an iota
  constant (building one-hotT tiles on chip), then a small-K matmul against
  host-prebuilt embedding tables (fc1_w columns re-indexed by action value).
  All biases are folded into the row-15 "ones" row of those tables.
- leaky_relu is decomposed as 0.1*x + 0.9*relu(x); the linear 0.1*x path is
  folded into the weights (Wfuse = 0.1*fc2_w@fc1_w) so the device only needs
  a single Relu per PSUM tile; the relu path uses 0.9-scaled fc2 weights.
- f32 mode uses float32r matmuls (full-rate PE); bf16 mode halves HBM bytes.

Outputs are produced transposed ([feat, rows]) and un-transposed on host.
"""
import os
import sys
import types

import numpy as np
import ml_dtypes

import concourse.bass as bass
import concourse.tile as tile
from concourse import bacc, mybir
from concourse.bass_utils import run_bass_kernel_spmd

# If BASS_TRACE is set in the environment, run_bass_kernel_spmd imports
# antenv.axon_hooks, which this image lacks. Provide a no-op hook module so
# tracing degrades gracefully instead of raising ImportError.
if "antenv.axon_hooks" not in sys.modules:
    try:
        import antenv.axon_hooks  # noqa: F401
    except ImportError:
        _m = types.ModuleType("antenv.axon_hooks")
        _m.get_axon_ntff_profile_hook = lambda: None
        _m.set_axon_ntff_profile_hook = lambda h: None
        sys.modules["antenv.axon_hooks"] = _m

# ---------------------------------------------------------------- constants
F_FRAMES, A_AGENTS, FEAT = 4096, 32, 128
NVEC = [5, 7, 9]
STARTS = [0, 5, 12]
ACT_TOTAL = 21            # sum(NVEC)
HID = 140
NCORES = 8
ROWS = F_FRAMES * A_AGENTS          # 131072
RPC = ROWS // NCORES                # 16384 rows per core
TILE_N = 1024                       # rows per megatile (2 PSUM banks)
NT = RPC // TILE_N                  # 16 megatiles per core
HID2 = HID - FEAT                   # 12 hidden dims beyond partition 128

# psum2 layout: rows 0:21 = logits, rows 21:32 = zero pad, rows 32:44 = h2
# (compute-engine PSUM reads must start at a 32-aligned partition)
NL = 44
H2_BASE = 32

# Variant: "bf16" (default, ~4e-3 scale-relative error),
#          "f32"  (float32 I/O, float32r matmuls — fast but pred path is
#                  less accurate than bf16 in absmax),
#          "f32x" (plain float32 matmuls — exact but PE runs at 1/4 rate)
VARIANT = os.environ.get("KERNEL_DTYPE", "bf16")

_prog_cache = {}


def _np_dt(variant):
    return ml_dtypes.bfloat16 if variant == "bf16" else np.float32


def _prep_weights(fc1_w, fc1_b, fc2_w, fc2_b,
                  head_w0, head_b0, head_w1, head_b1, head_w2, head_b2):
    """Build all stationary (lhsT) operands in float32. Shapes noted as [K, M]."""
    f32 = np.float32
    fc1_w = np.asarray(fc1_w, f32); fc1_b = np.asarray(fc1_b, f32)
    fc2_w = np.asarray(fc2_w, f32); fc2_b = np.asarray(fc2_b, f32)
    wcat = np.concatenate([np.asarray(head_w0, f32), np.asarray(head_w1, f32),
                           np.asarray(head_w2, f32)], axis=0)          # [21, 256]
    bcat = np.concatenate([np.asarray(head_b0, f32), np.asarray(head_b1, f32),
                           np.asarray(head_b2, f32)], axis=0)          # [21]

    wfuse = 0.1 * (fc2_w @ fc1_w)                                      # [128, 149]

    # embedding row index map: p = t*5 + v  ->  fc1 input column 128+STARTS[t]+v
    emb_cols = [FEAT + STARTS[t] + v for t in range(3) for v in range(5)]

    # Every lhsT is zero-padded to [128, 128]: uniform full-array matmuls
    # keep the PE pipeline dense (small-K / partial-M matmuls measured 2.8x
    # slower and prevent the clock gate from warming). Zero weight rows make
    # the padded contraction exact as long as the rhs rows are finite.
    w = {k: np.zeros((FEAT, FEAT), f32) for k in
         ("wa", "wb", "wc", "wd", "we", "wi", "wj", "wf2", "wg2")}
    w["wa"][:, :] = fc1_w[:FEAT, :FEAT].T               # h1 from cur
    w["wb"][:, :ACT_TOTAL] = wcat[:, :FEAT].T           # logits+h2 from cur
    w["wb"][:, H2_BASE:H2_BASE + HID2] = fc1_w[FEAT:, :FEAT].T
    w["wc"][:15, :] = fc1_w[:FEAT, emb_cols].T          # h1 from one-hot
    w["wc"][15, :] = fc1_b[:FEAT]
    w["wd"][:15, H2_BASE:H2_BASE + HID2] = fc1_w[FEAT:, emb_cols].T
    w["wd"][15, H2_BASE:H2_BASE + HID2] = fc1_b[FEAT:]
    w["wd"][15, :ACT_TOTAL] = bcat                      # h2 + all biases
    w["we"][:, :ACT_TOTAL] = wcat[:, FEAT:].T           # logits from nxt
    w["wi"][:, :] = wfuse[:, :FEAT].T                   # 0.1-path from cur
    w["wj"][:15, :] = wfuse[:, emb_cols].T              # 0.1-path from one-hot
    w["wj"][15, :] = 0.1 * (fc2_w @ fc1_b) + fc2_b
    w["wf2"][:, :] = (0.9 * fc2_w[:, :FEAT]).T          # relu path h1
    w["wg2"][H2_BASE:H2_BASE + HID2, :] = (0.9 * fc2_w[:, FEAT:]).T
    return w


def _build_program(variant):
    """Build the SPMD Bass program (identical across cores)."""
    if variant == "f32":
        dt_dat = mybir.dt.float32r     # matmul operands, stored as f32
        dt_out = mybir.dt.float32
    elif variant == "f32x":
        dt_dat = mybir.dt.float32
        dt_out = mybir.dt.float32
    else:
        dt_dat = mybir.dt.bfloat16
        dt_out = mybir.dt.bfloat16
    dt_act = mybir.dt.bfloat16         # action values (0..4, exact in bf16)
    f32 = mybir.dt.float32

    AF = mybir.ActivationFunctionType
    OP = mybir.AluOpType

    nc = bacc.Bacc(None, target_bir_lowering=False, debug=False)

    def din(name, shape, dt):
        return nc.dram_tensor(name, list(shape), dt, kind="ExternalInput").ap()

    def dout(name, shape, dt):
        return nc.dram_tensor(name, list(shape), dt, kind="ExternalOutput").ap()

    curT = din("curT", (FEAT, RPC), dt_dat)
    nxtT = din("nxtT", (FEAT, RPC), dt_dat)
    act16 = din("act16", (16, RPC), dt_act)
    w_names = ("wa", "wb", "wc", "wd", "we", "wi", "wj", "wf2", "wg2")
    w_all = din("w_all", (FEAT, FEAT * len(w_names)), dt_dat)
    iota128 = din("iota128", (128, 1), f32)
    predT = dout("predT", (FEAT, RPC), dt_out)
    logitsT = dout("logitsT", (ACT_TOTAL, RPC), f32)

    with tile.TileContext(nc) as tc:
        with (
            tc.tile_pool(name="consts", bufs=1) as cp,
            tc.tile_pool(name="io", bufs=4) as iop,
            tc.tile_pool(name="mid", bufs=3) as mp,
            tc.tile_pool(name="pp1", bufs=1, space="PSUM") as pp1,
            tc.tile_pool(name="pp2", bufs=1, space="PSUM") as pp2,
            tc.tile_pool(name="pp3", bufs=2, space="PSUM") as pp3,
        ):
            # ---- load constants once: scalar+gpsimd only, so the sync
            # engine can start the first tile's input DMAs immediately ----
            w_t = cp.tile([FEAT, FEAT * len(w_names)], dt_dat, name="c_wall")
            for q in range(4):
                ps_ = slice(q * 32, (q + 1) * 32)
                eng = nc.scalar if q % 2 == 0 else nc.gpsimd
                eng.dma_start(w_t[ps_, :], w_all[ps_, :])
            cw = {k: w_t[:, i * FEAT:(i + 1) * FEAT]
                  for i, k in enumerate(w_names)}
            iota_t = cp.tile([128, 1], f32, name="c_iota")
            nc.scalar.dma_start(iota_t[:], iota128)

            # variable tile schedule: small tiles at both ends shorten the
            # serial fill (first input transfer) and tail (last output chain)
            SCHED = [512, 512] + [1024] * 14 + [512, 512]
            NTV = len(SCHED)

            def emit_stage1(it, off, n):
                """input DMAs, one-hot build, and the 7 stage-1 matmuls"""
                sl = slice(off, off + n)
                cur_t = iop.tile([FEAT, n], dt_dat, tag="cur")
                if it == 0:
                    for q in range(4):
                        ps_ = slice(q * 32, (q + 1) * 32)
                        eng = nc.sync if q % 2 == 0 else nc.gpsimd
                        eng.dma_start(cur_t[ps_, :], curT[ps_, sl])
                else:
                    nc.sync.dma_start(cur_t[0:64, :], curT[0:64, sl])
                    nc.gpsimd.dma_start(cur_t[64:128, :], curT[64:128, sl])
                nxt_t = iop.tile([FEAT, n], dt_dat, tag="nxt")
                nc.gpsimd.dma_start(nxt_t[0:64, :], nxtT[0:64, sl])
                nc.sync.dma_start(nxt_t[64:128, :], nxtT[64:128, sl])
                act_t = iop.tile([128, n], dt_act, tag="act")
                nc.sync.dma_start(act_t[0:16, :], act16[:, sl])

                # one-hotT tile; only rows 0:16 are loaded — rows 16:127
                # compare stale SBUF data, but is_equal always yields finite
                # 0/1 and those rows hit zero-padded weight rows, so the
                # matmul contribution is exactly zero.
                oh = mp.tile([128, n], dt_dat, tag="oh")
                nc.vector.tensor_scalar(oh[:], act_t[:], iota_t[:], None,
                                        OP.is_equal)

                p1 = pp1.tile([FEAT, n], f32, tag="p1")   # h1 pre-act
                p2 = pp2.tile([FEAT, n], f32, tag="p2")   # logits + h2
                p3 = pp3.tile([FEAT, n], f32, tag="p3")   # pred accum
                # weight-major order: each stationary is used for both
                # 512-halves back-to-back so the weight load amortizes
                halves = [slice(s * 512, min((s + 1) * 512, n))
                          for s in range(max(1, n // 512))]
                for w_key, rhs, psum, st_, sp_ in (
                        ("wa", cur_t, p1, True, False),
                        ("wc", oh, p1, False, True),
                        ("wb", cur_t, p2, True, False),
                        ("wd", oh, p2, False, False),
                        ("we", nxt_t, p2, False, True),
                        ("wi", cur_t, p3, True, False),
                        ("wj", oh, p3, False, False)):
                    for hs in halves:
                        nc.tensor.matmul(psum[:, hs], cw[w_key], rhs[:, hs],
                                         start=st_, stop=sp_)

                h1 = mp.tile([FEAT, n], dt_dat, tag="h1")
                nc.scalar.activation(h1[:], p1[:], AF.Relu)
                hl2 = mp.tile([FEAT, n], dt_dat, tag="hl2")
                nc.scalar.activation(hl2[:], p2[:], AF.Relu)
                lg = mp.tile([ACT_TOTAL, n], f32, tag="lg")
                nc.vector.tensor_copy(lg[:], p2[0:ACT_TOTAL, :])
                return {"sl": sl, "p3": p3, "h1": h1, "hl2": hl2, "lg": lg,
                        "it": it, "n": n}

            def emit_stage2(st):
                """relu-path matmuls + pred/logits drain for a prior tile.

                Emitted after the NEXT tile's stage-1 matmuls so the PE never
                stalls waiting for this tile's Relu outputs (head-of-line
                blocking on the in-order PE queue)."""
                it, sl, p3 = st["it"], st["sl"], st["p3"]
                halves = [slice(s * 512, min((s + 1) * 512, st["n"]))
                          for s in range(max(1, st["n"] // 512))]
                for hs in halves:
                    nc.tensor.matmul(p3[:, hs], cw["wf2"], st["h1"][:, hs],
                                     start=False, stop=False)
                for hs in halves:
                    nc.tensor.matmul(p3[:, hs], cw["wg2"], st["hl2"][:, hs],
                                     start=False, stop=True)
                pred = mp.tile([FEAT, st["n"]], dt_out, tag="pred")
                nc.vector.tensor_copy(pred[:], p3[:])
                if it >= NTV - 2:
                    # last tiles: 4-way split so the tail DMA flush is short
                    nc.gpsimd.dma_start(predT[0:32, sl], pred[0:32, :])
                    nc.sync.dma_start(predT[32:64, sl], pred[32:64, :])
                    nc.gpsimd.dma_start(predT[64:96, sl], pred[64:96, :])
                    nc.sync.dma_start(predT[96:128, sl], pred[96:128, :])
                else:
                    nc.gpsimd.dma_start(predT[0:64, sl], pred[0:64, :])
                    nc.sync.dma_start(predT[64:128, sl], pred[64:128, :])
                if it >= NTV - 2:
                    nc.sync.dma_start(logitsT[:, sl], st["lg"][:])
                else:
                    nc.gpsimd.dma_start(logitsT[:, sl], st["lg"][:])

            prev = None
            off = 0
            for it, n in enumerate(SCHED):
                st = emit_stage1(it, off, n)
                off += n
                if prev is not None:
                    emit_stage2(prev)
                prev = st
            emit_stage2(prev)

    nc.compile()
    return nc


def _get_program(variant):
    if variant not in _prog_cache:
        _prog_cache[variant] = _build_program(variant)
    return _prog_cache[variant]


def _prep_inputs(inputs, variant):
    """Host-side data staging: transpose/shard/cast. Returns per-core in_maps."""
    npdt = _np_dt(variant)
    cur = np.asarray(inputs["current_feature"], np.float32).reshape(ROWS, FEAT)
    nxt = np.asarray(inputs["next_feature"], np.float32).reshape(ROWS, FEAT)
    acts = np.asarray(inputs["actions"])                       # [F, 3, A] int32

    curT = np.ascontiguousarray(cur.T).astype(npdt)            # [128, ROWS]
    nxtT = np.ascontiguousarray(nxt.T).astype(npdt)
    a3 = np.transpose(acts, (1, 0, 2)).reshape(3, ROWS)        # row t = type t
    act16 = np.zeros((16, ROWS), ml_dtypes.bfloat16)
    act16[:15] = np.repeat(a3, 5, axis=0).astype(ml_dtypes.bfloat16)

    w = _prep_weights(
        inputs["fc1_w"], inputs["fc1_b"], inputs["fc2_w"], inputs["fc2_b"],
        inputs["head_w0"], inputs["head_b0"], inputs["head_w1"],
        inputs["head_b1"], inputs["head_w2"], inputs["head_b2"])
    w_names = ("wa", "wb", "wc", "wd", "we", "wi", "wj", "wf2", "wg2")
    w_all = np.ascontiguousarray(
        np.concatenate([w[k] for k in w_names], axis=1)).astype(npdt)
    w = {"w_all": w_all}
    iota128 = np.ascontiguousarray(
        np.tile(np.array([0, 1, 2, 3, 4] * 3 + [0], np.float32), 8)
    ).reshape(128, 1)

    in_maps = []
    for c in range(NCORES):
        s = slice(c * RPC, (c + 1) * RPC)
        m = {"curT": np.ascontiguousarray(curT[:, s]),
             "nxtT": np.ascontiguousarray(nxtT[:, s]),
             "act16": np.ascontiguousarray(act16[:, s]),
             "iota128": iota128}
        m.update(w)
        in_maps.append(m)
    return in_maps


def _assemble_outputs(results):
    predT = np.concatenate([np.asarray(r["predT"], np.float32)
                            for r in results], axis=1)          # [128, ROWS]
    logitsT = np.concatenate([np.asarray(r["logitsT"], np.float32)
                              for r in results], axis=1)        # [21, ROWS]
    pred = predT.T.reshape(F_FRAMES, A_AGENTS, FEAT).astype(np.float32)
    lg = logitsT.T                                              # [ROWS, 21]
    logits0 = lg[:, 0:5].reshape(F_FRAMES, A_AGENTS, 5).astype(np.float32)
    logits1 = lg[:, 5:12].reshape(F_FRAMES, A_AGENTS, 7).astype(np.float32)
    logits2 = lg[:, 12:21].reshape(F_FRAMES, A_AGENTS, 9).astype(np.float32)
    return pred, logits0, logits1, logits2


def kernel(**inputs):
    variant = VARIANT
    nc = _get_program(variant)
    in_maps = _prep_inputs(inputs, variant)
    res = run_bass_kernel_spmd(nc, in_maps, list(range(NCORES)))
    return _assemble_outputs(res.results)
